# revision 32
# baseline (speedup 1.0000x reference)
"""Bass/Tile TRN2 kernel for nn_Attention (additive/Bahdanau-style attention).

reference math per batch b:
  res_q = query[b] @ W_q.T                      (Q, H)
  res_c = context[b] @ W_c.T + b_c              (C, H)
  logit[q,c] = sum_h W_o[h]*tanh(res_c[c,h] + res_q[q,h]) + b_o
  w = mask * exp(logit); weights = w / (sum_c w + eps)
  out = weights @ context[b]

The (Q,C,H) tanh grid is never materialized. tanh is replaced by an
M-term harmonic sine series  tanh(x) ~= k*x + sum_m c_m sin(m*w0*x), and
the angle-addition identity factorizes each term so the whole logit is
ONE PE contraction:  logit[q,c] = sum_f B_f[.,q] * A_f[.,c]   where
 - the harmonic features contract over (h, m, sin/cos) chunks of 128:
   only the fundamental sin/cos touch the ACT Sin table (cos through one
   DVE add_range_wrap with the pi/2 phase folded into the wrap shift);
   harmonic 2 comes from double-angle products, with the 2x of
   s2'=sin2/2 folded into the host-side W_o*c_m coefficients,
 - b_c folds into the rq staging copy (per-partition DVE bias), so the
   A(context) side is bias-free,
 - the linear k*Wo.rc term contracts over d against ctxT directly via
   the host-precomputed u = k*(Wo @ W_c), so res_c is never staged to
   bf16 (the sins read the f32 PSUM in place),
 - the linear k*Wo.rq' term broadcasts over c via a ones moving tile,
 - a rank-1 ln(mask) chunk folds the mask into the logit so exp's
   accum_out directly yields the masked softmax denominator.

The PE transposes of the raw exp start right after exp (no rowsum
dependency); 1/rowsum lands as a per-partition ACT scale on the final
output copy.

All matmul operands are bf16; PSUM accumulation stays f32.  Every bulk
input is pre-rearranged on the host into a [128, n*cols] partition-major
layout so each DMA moves few large contiguous lines.  Both HW-DGE
queues share one ~200GB/s DMA engine, so bulk rides the sync queue with
the CONTEXT side first: the critical chain is ctxT -> res_c -> sins ->
products -> last contraction chunks -> exp -> out, while the small
query side streams later and slots into the gaps.

Sharding: data-parallel over batch B=8 across the 8 NeuronCores.
"""

import numpy as np

B, Q, C, D, H = 8, 64, 512, 512, 256
EPS = 1e-5
P = 128
KD = D // P   # 4 chunks of the contraction dim d
KC = C // P   # 4 chunks of the context dim c
JH = H // P   # 2 chunks of the hidden dim h
N_CORES = 8

# tanh(x) ~= K_LIN*x + sum_m CS[m]*sin(m*W0*x), fitted on [-4.7, 4.7]
M_HARM = 2
W0 = 1.05
K_LIN = 0.3266410020214013
CS = [0.4380670801317152, 0.07800815282640118]
PI = float(np.pi)
TRIM = 1.0 - 5e-7    # keeps |w0*x| strictly inside the Sin table domain


def _build_program(b_o_val: float):
    import concourse.bacc as bacc
    import concourse.mybir as mybir
    import concourse.tile as tile
    from concourse.alu_op_type import AluOpType
    from concourse import masks
    from contextlib import ExitStack

    F32 = mybir.dt.float32
    BF16 = mybir.dt.bfloat16
    Act = mybir.ActivationFunctionType

    nc = bacc.Bacc("TRN2", target_bir_lowering=False, debug=False)

    # all bulk inputs arrive pre-rearranged: [P, k*cols], partition-major
    # cbulk = [WcT (KD*H) | ctxT (KD*C)], qbulk = [qT (KD*Q) | WqT (KD*H)]:
    # packing lets the whole context stream ride TWO sync triggers and the
    # query stream ONE, instead of five serialized trigger issues
    cbulk_d = nc.dram_tensor("cbulk", [P, KD * (H + C)], BF16, kind="ExternalInput")
    qbulk_d = nc.dram_tensor("qbulk", [P, KD * (Q + H)], BF16, kind="ExternalInput")
    ctx_d = nc.dram_tensor("ctx", [P, KC * D], BF16, kind="ExternalInput")
    mrow_d = nc.dram_tensor("mrow", [1, C], BF16, kind="ExternalInput")
    # cols 0..M-1: W_o * c_m folds (2x on m=2 for the halved sin2 product);
    # col M: W_o * k_lin; col M+1: b_c
    WoCK_d = nc.dram_tensor("WoCK", [P, JH, M_HARM + 2], F32, kind="ExternalInput")
    # u2[p, k] = (k_lin * W_o @ W_c)[k*128+p]: the linear context term
    # contracts over d against ctxT directly
    u2_d = nc.dram_tensor("u2", [P, KD], F32, kind="ExternalInput")
    out_d = nc.dram_tensor("out", [Q, D], BF16, kind="ExternalOutput")
    wts_d = nc.dram_tensor("wts", [Q, C], BF16, kind="ExternalOutput")

    with tile.TileContext(nc) as tc, ExitStack() as ctx:
        const = ctx.enter_context(tc.tile_pool(name="const", bufs=1))
        sm = ctx.enter_context(tc.tile_pool(name="sm", bufs=1))
        ps_rc = ctx.enter_context(tc.tile_pool(name="ps_rc", bufs=1, space="PSUM"))
        ps_rq = ctx.enter_context(tc.tile_pool(name="ps_rq", bufs=1, space="PSUM"))
        ps_lg = ctx.enter_context(tc.tile_pool(name="ps_lg", bufs=1, space="PSUM"))
        ps_tp = ctx.enter_context(tc.tile_pool(name="ps_tp", bufs=1, space="PSUM"))
        ps_ou = ctx.enter_context(tc.tile_pool(name="ps_ou", bufs=1, space="PSUM"))

        # ---- input DMAs: context side first on the sync queue
        cbulk_sb = const.tile([P, KD * (H + C)], BF16)
        qbulk_sb = const.tile([P, KD * (Q + H)], BF16)
        WoCK_sb = const.tile([P, JH, M_HARM + 2], F32)
        u2_sb = const.tile([P, KD], F32)
        mrow_sb = const.tile([1, C], BF16)
        CB0 = KD * H   # ctxT base inside cbulk
        QB0 = KD * Q   # WqT base inside qbulk

        def WcTa(k, hs):
            return cbulk_sb[:, k * H + hs.start : k * H + hs.stop]

        def ctxTa(k):
            return cbulk_sb[:, CB0 + k * C : CB0 + (k + 1) * C]

        def qTa(k):
            return qbulk_sb[:, k * Q : (k + 1) * Q]

        def WqTa(k, hs):
            return qbulk_sb[:, QB0 + k * H + hs.start : QB0 + k * H + hs.stop]

        split = CB0 + 2 * C
        nc.sync.dma_start(cbulk_sb[:, 0:split], cbulk_d.ap()[:, 0:split])
        nc.sync.dma_start(
            cbulk_sb[:, split : KD * (H + C)], cbulk_d.ap()[:, split : KD * (H + C)]
        )
        nc.sync.dma_start(qbulk_sb[:], qbulk_d.ap())
        nc.scalar.dma_start(WoCK_sb[:], WoCK_d.ap())
        nc.scalar.dma_start(u2_sb[:], u2_d.ap())
        nc.scalar.dma_start(mrow_sb[:], mrow_d.ap())
        # ctx (only needed by the final weights @ ctx matmul) is triggered
        # after the res matmuls are emitted: readers of any earlier tensor
        # conservatively wait on all previously-issued DMAs
        ctx_sb = const.tile([P, KC, D], BF16)

        onesC = const.tile([P, C], BF16)
        nc.gpsimd.memset(onesC[:], 1.0)
        onesQ = const.tile([P, Q], BF16)
        nc.gpsimd.memset(onesQ[:], 1.0)
        ident = const.tile([Q, Q], F32)
        masks.make_identity(nc, ident[:])
        bo_sb = const.tile([P, 1], F32)
        nc.vector.memset(bo_sb[:], float(b_o_val))
        # dummy first ACT op: forces the trig table (which also contains
        # Identity/Copy/Square) to be the one loaded during the DMA stream
        warm = const.tile([P, Q], BF16)
        nc.vector.memset(warm[:], 0.25)
        sinwarm = sm.tile([1, 1], BF16, name="sinwarm")
        nc.scalar.activation(sinwarm[:], warm[0:1, 0:1], Act.Sin, bias=0.0, scale=W0)
        # Bu[p, k, q] = u2[p, k] broadcast over q (stationary for the
        # linear-context chunks)
        Bu = sm.tile([P, KD, Q], BF16, name="Bu")
        for k in range(KD):
            nc.vector.tensor_scalar(
                Bu[:, k, :], onesQ[:], u2_sb[:, k : k + 1], None, AluOpType.mult
            )

        # ---- PE warmup junk while the context stream lands; then res_c
        # (k-outer, chasing the two ctxT DMA chunks) interleaved with the
        # linear-context lg chunks, then res_q as soon as its inputs land.
        rcp = ps_rc.tile([P, JH, C], F32)
        rqp = ps_rq.tile([P, JH, Q], F32)
        tp = ps_tp.tile([P, KC, Q], F32)
        lg = ps_lg.tile([Q, C], F32)
        for i in range(56):
            nc.tensor.matmul(
                tp[0:Q, 0, :], warm[:], warm[:], start=True, stop=True
            )
        first = dict(v=True)

        def mm(bt, at, stop=False):
            nc.tensor.matmul(lg[:], bt, at, start=first["v"], stop=stop)
            first["v"] = False

        for k in range(KD):
            for j in range(JH):
                hs = slice(j * P, (j + 1) * P)
                nc.tensor.matmul(
                    rcp[:, j, :], WcTa(k, hs), ctxTa(k),
                    start=(k == 0), stop=(k == KD - 1),
                )
            if k == 1:
                # res_q slots into the ctxT-k23 DMA wait: the B chain then
                # runs on ACT/DVE before the A sins are even ready
                for j in range(JH):
                    hs = slice(j * P, (j + 1) * P)
                    for kq in range(KD):
                        nc.tensor.matmul(
                            rqp[:, j, :], WqTa(kq, hs), qTa(kq),
                            start=(kq == 0), stop=(kq == KD - 1),
                        )
        # linear-context chunks AFTER res_c: they fill the PE during the
        # ACT sin phase instead of delaying res_c's last matmul
        for k in range(KD):
            mm(Bu[:, k, :], ctxTa(k))               # k_lin*Wo.rc over d
        nc.sync.dma_start(ctx_sb[:], ctx_d.ap())

        # ---- A side (context, 512 cols): the sins read the res_c PSUM
        # directly, split per h-chunk j so the PE's m0 chunks start one
        # sin earlier; products on the DVE.
        P0 = 2.0 * PI / W0
        wA = sm.tile([P, JH, C], F32, name="wA")
        sA = sm.tile([P, JH, C], BF16, name="sA")
        cA = sm.tile([P, JH, C], BF16, name="cA")
        s1sA = sm.tile([P, JH, C], BF16, name="s1sA")
        s2A = sm.tile([P, JH, C], BF16, name="s2A")
        c2A = sm.tile([P, JH, C], BF16, name="c2A")
        # B side first: it only needs res_q, which completed mid-res_c,
        # so the whole B chain clears ACT/DVE before the A sins are ready
        rq_sb = sm.tile([P, JH, Q], BF16, name="rq_sb")
        for j in range(JH):
            nc.vector.tensor_scalar(
                rq_sb[:, j, :], rqp[:, j, :],
                WoCK_sb[:, j, M_HARM + 1 : M_HARM + 2], None, AluOpType.add,
            )
        wB = sm.tile([P, JH, Q], F32, name="wB")
        nc.vector.add_range_wrap(wB[:], rq_sb[:], (PI / 2) / W0, P0 / 2, P0)
        sB = sm.tile([P, JH, Q], BF16, name="sB")
        cB = sm.tile([P, JH, Q], BF16, name="cB")
        nc.scalar.activation(sB[:], rq_sb[:], Act.Sin, bias=0.0, scale=W0 * TRIM)
        nc.scalar.activation(cB[:], wB[:], Act.Sin, bias=0.0, scale=W0 * TRIM)
        BlinQ = sm.tile([P, JH, Q], BF16, name="BlinQ")
        for j in range(JH):
            nc.vector.tensor_scalar(
                BlinQ[:, j, :], rq_sb[:, j, :],
                WoCK_sb[:, j, M_HARM : M_HARM + 1], None, AluOpType.mult,
            )
        gB = sm.tile([P, M_HARM, 2, JH, Q], BF16, name="gB")

        def fold(m, t, src):
            for j in range(JH):
                nc.vector.tensor_scalar(
                    gB[:, m, t, j, :], src[:, j, :],
                    WoCK_sb[:, j, m : m + 1], None, AluOpType.mult,
                )

        fold(0, 0, sB)
        fold(0, 1, cB)
        s1sB = sm.tile([P, JH, Q], BF16, name="s1sB")
        nc.vector.tensor_tensor(s1sB[:], sB[:], sB[:], AluOpType.mult)
        s2B = sm.tile([P, JH, Q], BF16, name="s2B")
        nc.vector.tensor_tensor(s2B[:], sB[:], cB[:], AluOpType.mult)
        c2B = sm.tile([P, JH, Q], BF16, name="c2B")
        nc.vector.tensor_scalar(c2B[:], s1sB[:], -2.0, 1.0, AluOpType.mult, AluOpType.add)
        fold(1, 0, s2B)
        fold(1, 1, c2B)

        # A side: wraps + the four A sins back-to-back, products chasing
        for j in range(JH):
            nc.vector.add_range_wrap(
                wA[:, j, :], rcp[:, j, :], (PI / 2) / W0, P0 / 2, P0
            )
        nc.scalar.activation(
            sA[:, 0, :], rcp[:, 0, :], Act.Sin, bias=0.0, scale=W0 * TRIM
        )
        nc.scalar.activation(
            cA[:, 0, :], wA[:, 0, :], Act.Sin, bias=0.0, scale=W0 * TRIM
        )
        nc.scalar.activation(
            sA[:, 1, :], rcp[:, 1, :], Act.Sin, bias=0.0, scale=W0 * TRIM
        )
        nc.scalar.activation(
            cA[:, 1, :], wA[:, 1, :], Act.Sin, bias=0.0, scale=W0 * TRIM
        )

        def a_products(j):
            # s1s needs only sA (early); c2 needs only s1s -- both run
            # before s2 (which waits for the late cA), so the DVE finishes
            # the final feature as soon as cA lands
            nc.vector.tensor_tensor(
                s1sA[:, j, :], sA[:, j, :], sA[:, j, :], AluOpType.mult
            )
            nc.vector.tensor_scalar(
                c2A[:, j, :], s1sA[:, j, :], -2.0, 1.0, AluOpType.mult, AluOpType.add
            )
            nc.vector.tensor_tensor(
                s2A[:, j, :], sA[:, j, :], cA[:, j, :], AluOpType.mult
            )

        a_products(0)
        a_products(1)

        # ---- remaining logit chunks, ordered by feature readiness
        mm(onesQ[0:1, :], mrow_sb[:])               # ln(mask) rank-1
        for j in range(JH):
            mm(BlinQ[:, j, :], onesC[:])            # k*Wo.rq' broadcast over c
        for i in range(26):  # p-state bridge over the feature wait
            nc.tensor.matmul(
                tp[0:Q, 1, :], warm[:], warm[:], start=True, stop=True
            )
        mm(gB[:, 0, 1, 0, :], sA[:, 0, :])
        mm(gB[:, 1, 0, 0, :], c2A[:, 0, :])
        mm(gB[:, 0, 0, 0, :], cA[:, 0, :])
        mm(gB[:, 1, 1, 0, :], s2A[:, 0, :])
        mm(gB[:, 0, 1, 1, :], sA[:, 1, :])
        mm(gB[:, 1, 0, 1, :], c2A[:, 1, :])
        mm(gB[:, 0, 0, 1, :], cA[:, 1, :])
        mm(gB[:, 1, 1, 1, :], s2A[:, 1, :], stop=True)

        # ---- softmax tail: exp (+ masked row sums via accum_out), PE
        # transposes of the raw exp start immediately (no rowsum wait); the
        # 1/rowsum lands as a per-partition ACT scale on the final copy.
        expQ = sm.tile([Q, C], F32)
        sumQ = sm.tile([Q, 1], F32)
        nc.scalar.activation(
            expQ[:], lg[:], Act.Exp, bias=bo_sb[0:Q, 0:1], accum_out=sumQ[:]
        )
        for i in range(5):  # keep the PE p-state up through the exp wait
            nc.tensor.matmul(
                tp[0:Q, 0, :], warm[:], warm[:], start=True, stop=True
            )
        for k in range(KC):
            nc.tensor.transpose(
                tp[:, k, :], expQ[:, k * P : (k + 1) * P], ident[:]
            )
        eT_sb = sm.tile([P, KC, Q], BF16)
        nc.scalar.activation(eT_sb[:], tp[:], Act.Copy)
        # recQ chain emitted after eT so its ACT-side accumulator read does
        # not delay the eT staging on the in-order ACT queue
        recQ = sm.tile([Q, 1], F32)
        nc.vector.tensor_scalar_add(recQ[:], sumQ[:], float(EPS))
        nc.vector.reciprocal(recQ[:], recQ[:])
        w_sb = sm.tile([Q, C], BF16)
        nc.vector.tensor_scalar(
            w_sb[:], expQ[:], recQ[:, 0:1], None, AluOpType.mult
        )
        nc.sync.dma_start(wts_d.ap()[:, :], w_sb[:])
        ou = ps_ou.tile([Q, D], F32)
        for i in range(3):  # bridge the eT staging wait on a warm PE
            nc.tensor.matmul(
                tp[0:Q, 1, :], warm[:], warm[:], start=True, stop=True
            )
        for k in range(KC):
            nc.tensor.matmul(
                ou[:], eT_sb[:, k, :], ctx_sb[:, k, :],
                start=(k == 0), stop=(k == KC - 1),
            )
        out_sb = sm.tile([Q, D], BF16)
        nc.scalar.activation(out_sb[:], ou[:], Act.Copy, scale=recQ[:, 0:1])
        nc.sync.dma_start(out_d.ap()[:, :], out_sb[:])

    nc.compile()
    return nc


def _chunked(a, p=P):
    """[N*p, cols] -> [p, N*cols]: row k*p+i lands at [i, k*cols:(k+1)*cols]."""
    n = a.shape[0] // p
    return np.ascontiguousarray(
        a.reshape(n, p, a.shape[1]).transpose(1, 0, 2).reshape(p, n * a.shape[1])
    )


def make_in_maps(query, context, mask, W_c, b_c, W_q, W_o):
    import ml_dtypes
    f32 = np.float32
    bf16 = ml_dtypes.bfloat16
    WqT = _chunked(np.asarray(W_q, f32).T.astype(bf16))
    WcT = _chunked(np.asarray(W_c, f32).T.astype(bf16))
    qbulk_w = WqT  # appended after qT per batch below
    Wo2 = np.asarray(W_o, f32).reshape(JH, P).T  # (P, JH)
    # the m=2 product feature is sin2/2, so its fold carries 2x
    cols = [f32(c) for c in CS]
    cols[1] = f32(2.0) * cols[1]
    cols.append(f32(K_LIN))
    WoCK = np.stack([Wo2 * c for c in cols], axis=2)  # (P, JH, M+1)
    bc2 = np.asarray(b_c, f32).reshape(JH, P).T[:, :, None]  # (P, JH, 1)
    WoCKB = np.ascontiguousarray(
        np.concatenate([WoCK, bc2], axis=2).astype(f32)
    )  # (P, JH, M+2)
    u = f32(K_LIN) * (np.asarray(W_o, f32) @ np.asarray(W_c, f32))  # (D,)
    u2 = np.ascontiguousarray(u.reshape(KD, P).T.astype(f32))  # (P, KD)
    in_maps = []
    for b in range(B):
        mrow = np.asarray(mask[b], f32)
        mbr = np.maximum(np.log(np.maximum(mrow, 1e-300)), -50.0)
        in_maps.append(
            {
                "cbulk": np.ascontiguousarray(np.concatenate(
                    [WcT, _chunked(np.asarray(context[b], f32).T.astype(bf16))],
                    axis=1,
                )),
                "qbulk": np.ascontiguousarray(np.concatenate(
                    [_chunked(np.asarray(query[b], f32).T.astype(bf16)), qbulk_w],
                    axis=1,
                )),
                "ctx": _chunked(np.asarray(context[b], bf16)),
                "mrow": np.ascontiguousarray(mbr.reshape(1, C).astype(bf16)),
                "WoCK": WoCKB,
                "u2": u2,
            }
        )
    return in_maps


def kernel(query, context, mask, W_c, b_c, W_q, W_o, b_o):
    from concourse.bass_utils import run_bass_kernel_spmd

    nc = _build_program(float(np.asarray(b_o)))
    in_maps = make_in_maps(query, context, mask, W_c, b_c, W_q, W_o)
    res = run_bass_kernel_spmd(nc, in_maps, list(range(N_CORES))).results
    out = np.stack([np.asarray(res[b]["out"], np.float32) for b in range(B)])
    wts = np.stack([np.asarray(res[b]["wts"], np.float32) for b in range(B)])
    return out, wts


# revision 33
# speedup vs baseline: 1.0254x; 1.0254x over previous
"""Bass/Tile TRN2 kernel for nn_Attention (additive/Bahdanau-style attention).

reference math per batch b:
  res_q = query[b] @ W_q.T                      (Q, H)
  res_c = context[b] @ W_c.T + b_c              (C, H)
  logit[q,c] = sum_h W_o[h]*tanh(res_c[c,h] + res_q[q,h]) + b_o
  w = mask * exp(logit); weights = w / (sum_c w + eps)
  out = weights @ context[b]

The (Q,C,H) tanh grid is never materialized. tanh is replaced by an
M-term harmonic sine series  tanh(x) ~= k*x + sum_m c_m sin(m*w0*x), and
the angle-addition identity factorizes each term so the whole logit is
ONE PE contraction:  logit[q,c] = sum_f B_f[.,q] * A_f[.,c]   where
 - the harmonic features contract over (h, m, sin/cos) chunks of 128:
   only the fundamental sin/cos touch the ACT Sin table (cos through one
   DVE add_range_wrap with the pi/2 phase folded into the wrap shift);
   harmonic 2 comes from double-angle products, with the 2x of
   s2'=sin2/2 folded into the host-side W_o*c_m coefficients,
 - b_c folds into the rq staging copy (per-partition DVE bias), so the
   A(context) side is bias-free,
 - the linear k*Wo.rc term contracts over d against ctxT directly via
   the host-precomputed u = k*(Wo @ W_c), so res_c is never staged to
   bf16 (the sins read the f32 PSUM in place),
 - the linear k*Wo.rq' term broadcasts over c via a ones moving tile,
 - a rank-1 ln(mask) chunk folds the mask into the logit so exp's
   accum_out directly yields the masked softmax denominator.

The PE transposes of the raw exp start right after exp (no rowsum
dependency); 1/rowsum lands as a per-partition ACT scale on the final
output copy.

All matmul operands are bf16; PSUM accumulation stays f32.  Every bulk
input is pre-rearranged on the host into a [128, n*cols] partition-major
layout so each DMA moves few large contiguous lines.  Both HW-DGE
queues share one ~200GB/s DMA engine, so bulk rides the sync queue with
the CONTEXT side first: the critical chain is ctxT -> res_c -> sins ->
products -> last contraction chunks -> exp -> out, while the small
query side streams later and slots into the gaps.

Sharding: data-parallel over batch B=8 across the 8 NeuronCores.
"""

import numpy as np

B, Q, C, D, H = 8, 64, 512, 512, 256
EPS = 1e-5
P = 128
KD = D // P   # 4 chunks of the contraction dim d
KC = C // P   # 4 chunks of the context dim c
JH = H // P   # 2 chunks of the hidden dim h
N_CORES = 8

# tanh(x) ~= K_LIN*x + sum_m CS[m]*sin(m*W0*x), fitted on [-4.7, 4.7]
M_HARM = 2
W0 = 1.05
K_LIN = 0.3266410020214013
CS = [0.4380670801317152, 0.07800815282640118]
PI = float(np.pi)
TRIM = 1.0 - 5e-7    # keeps |w0*x| strictly inside the Sin table domain


def _build_program(b_o_val: float):
    import concourse.bacc as bacc
    import concourse.mybir as mybir
    import concourse.tile as tile
    from concourse.alu_op_type import AluOpType
    from concourse import masks
    from contextlib import ExitStack

    F32 = mybir.dt.float32
    BF16 = mybir.dt.bfloat16
    Act = mybir.ActivationFunctionType

    nc = bacc.Bacc("TRN2", target_bir_lowering=False, debug=False)

    # all bulk inputs arrive pre-rearranged: [P, k*cols], partition-major
    # cbulk = [WcT (KD*H) | ctxT (KD*C)], qbulk = [qT (KD*Q) | WqT (KD*H)]:
    # packing lets the whole context stream ride TWO sync triggers and the
    # query stream ONE, instead of five serialized trigger issues
    cbulk_d = nc.dram_tensor("cbulk", [P, KD * (H + C)], BF16, kind="ExternalInput")
    qbulk_d = nc.dram_tensor("qbulk", [P, KD * (Q + H)], BF16, kind="ExternalInput")
    ctx_d = nc.dram_tensor("ctx", [P, KC * D], BF16, kind="ExternalInput")
    mrow_d = nc.dram_tensor("mrow", [1, C], BF16, kind="ExternalInput")
    # cols 0..M-1: W_o * c_m folds (2x on m=2 for the halved sin2 product);
    # col M: W_o * k_lin; col M+1: b_c
    WoCK_d = nc.dram_tensor("WoCK", [P, JH, M_HARM + 2], F32, kind="ExternalInput")
    # u2[p, k] = (k_lin * W_o @ W_c)[k*128+p]: the linear context term
    # contracts over d against ctxT directly
    u2_d = nc.dram_tensor("u2", [P, KD], F32, kind="ExternalInput")
    out_d = nc.dram_tensor("out", [Q, D], BF16, kind="ExternalOutput")
    wts_d = nc.dram_tensor("wts", [Q, C], BF16, kind="ExternalOutput")

    with tile.TileContext(nc) as tc, ExitStack() as ctx:
        const = ctx.enter_context(tc.tile_pool(name="const", bufs=1))
        sm = ctx.enter_context(tc.tile_pool(name="sm", bufs=1))
        ps_rc = ctx.enter_context(tc.tile_pool(name="ps_rc", bufs=1, space="PSUM"))
        ps_rq = ctx.enter_context(tc.tile_pool(name="ps_rq", bufs=1, space="PSUM"))
        ps_lg = ctx.enter_context(tc.tile_pool(name="ps_lg", bufs=1, space="PSUM"))
        ps_tp = ctx.enter_context(tc.tile_pool(name="ps_tp", bufs=1, space="PSUM"))
        ps_ou = ctx.enter_context(tc.tile_pool(name="ps_ou", bufs=1, space="PSUM"))

        # ---- input DMAs: context side first on the sync queue
        cbulk_sb = const.tile([P, KD * (H + C)], BF16)
        qbulk_sb = const.tile([P, KD * (Q + H)], BF16)
        WoCK_sb = const.tile([P, JH, M_HARM + 2], F32)
        u2_sb = const.tile([P, KD], F32)
        mrow_sb = const.tile([1, C], BF16)
        CB0 = KD * H   # ctxT base inside cbulk
        QB0 = KD * Q   # WqT base inside qbulk

        def WcTa(k, hs):
            return cbulk_sb[:, k * H + hs.start : k * H + hs.stop]

        def ctxTa(k):
            return cbulk_sb[:, CB0 + k * C : CB0 + (k + 1) * C]

        def qTa(k):
            return qbulk_sb[:, k * Q : (k + 1) * Q]

        def WqTa(k, hs):
            return qbulk_sb[:, QB0 + k * H + hs.start : QB0 + k * H + hs.stop]

        split = CB0 + 2 * C
        nc.sync.dma_start(cbulk_sb[:, 0:split], cbulk_d.ap()[:, 0:split])
        nc.sync.dma_start(
            cbulk_sb[:, split : KD * (H + C)], cbulk_d.ap()[:, split : KD * (H + C)]
        )
        nc.sync.dma_start(qbulk_sb[:], qbulk_d.ap())
        nc.scalar.dma_start(WoCK_sb[:], WoCK_d.ap())
        nc.scalar.dma_start(u2_sb[:], u2_d.ap())
        nc.scalar.dma_start(mrow_sb[:], mrow_d.ap())
        # ctx (only needed by the final weights @ ctx matmul) is triggered
        # after the res matmuls are emitted: readers of any earlier tensor
        # conservatively wait on all previously-issued DMAs
        ctx_sb = const.tile([P, KC, D], BF16)

        onesC = const.tile([P, C], BF16)
        nc.gpsimd.memset(onesC[:], 1.0)
        onesQ = const.tile([P, Q], BF16)
        nc.gpsimd.memset(onesQ[:], 1.0)
        ident = const.tile([Q, Q], F32)
        masks.make_identity(nc, ident[:])
        bo_sb = const.tile([P, 1], F32)
        nc.vector.memset(bo_sb[:], float(b_o_val))
        # dummy first ACT op: forces the trig table (which also contains
        # Identity/Copy/Square) to be the one loaded during the DMA stream
        warm = const.tile([P, Q], BF16)
        nc.vector.memset(warm[:], 0.25)
        sinwarm = sm.tile([1, 1], BF16, name="sinwarm")
        nc.scalar.activation(sinwarm[:], warm[0:1, 0:1], Act.Sin, bias=0.0, scale=W0)
        # Bu[p, k, q] = u2[p, k] broadcast over q (stationary for the
        # linear-context chunks)
        Bu = sm.tile([P, KD, Q], BF16, name="Bu")
        for k in range(KD):
            nc.vector.tensor_scalar(
                Bu[:, k, :], onesQ[:], u2_sb[:, k : k + 1], None, AluOpType.mult
            )

        # ---- PE warmup junk while the context stream lands; then res_c
        # (k-outer, chasing the two ctxT DMA chunks) interleaved with the
        # linear-context lg chunks, then res_q as soon as its inputs land.
        rcp = ps_rc.tile([P, JH, C], F32)
        rqp = ps_rq.tile([P, JH, Q], F32)
        tp = ps_tp.tile([P, KC, Q], F32)
        lg = ps_lg.tile([Q, C], F32)
        for i in range(56):
            nc.tensor.matmul(
                tp[0:Q, 0, :], warm[:], warm[:], start=True, stop=True
            )
        first = dict(v=True)

        def mm(bt, at, stop=False):
            nc.tensor.matmul(lg[:], bt, at, start=first["v"], stop=stop)
            first["v"] = False

        for k in range(KD):
            for j in range(JH):
                hs = slice(j * P, (j + 1) * P)
                nc.tensor.matmul(
                    rcp[:, j, :], WcTa(k, hs), ctxTa(k),
                    start=(k == 0), stop=(k == KD - 1),
                )
            if k == 1:
                # res_q slots into the ctxT-k23 DMA wait: the B chain then
                # runs on ACT/DVE before the A sins are even ready
                for j in range(JH):
                    hs = slice(j * P, (j + 1) * P)
                    for kq in range(KD):
                        nc.tensor.matmul(
                            rqp[:, j, :], WqTa(kq, hs), qTa(kq),
                            start=(kq == 0), stop=(kq == KD - 1),
                        )
        # linear-context chunks AFTER res_c: they fill the PE during the
        # ACT sin phase instead of delaying res_c's last matmul
        for k in range(KD):
            mm(Bu[:, k, :], ctxTa(k))               # k_lin*Wo.rc over d
        nc.sync.dma_start(ctx_sb[:], ctx_d.ap())

        # ---- A side (context, 512 cols): the sins read the res_c PSUM
        # directly, split per h-chunk j so the PE's m0 chunks start one
        # sin earlier; products on the DVE.
        P0 = 2.0 * PI / W0
        wA = sm.tile([P, JH, C], F32, name="wA")
        sA = sm.tile([P, JH, C], BF16, name="sA")
        cA = sm.tile([P, JH, C], BF16, name="cA")
        s1sA = sm.tile([P, JH, C], BF16, name="s1sA")
        s2A = sm.tile([P, JH, C], BF16, name="s2A")
        c2A = sm.tile([P, JH, C], BF16, name="c2A")
        # B side first: it only needs res_q, which completed mid-res_c,
        # so the whole B chain clears ACT/DVE before the A sins are ready
        rq_sb = sm.tile([P, JH, Q], BF16, name="rq_sb")
        for j in range(JH):
            nc.vector.tensor_scalar(
                rq_sb[:, j, :], rqp[:, j, :],
                WoCK_sb[:, j, M_HARM + 1 : M_HARM + 2], None, AluOpType.add,
            )
        wB = sm.tile([P, JH, Q], F32, name="wB")
        nc.vector.add_range_wrap(wB[:], rq_sb[:], (PI / 2) / W0, P0 / 2, P0)
        sB = sm.tile([P, JH, Q], BF16, name="sB")
        cB = sm.tile([P, JH, Q], BF16, name="cB")
        nc.scalar.activation(sB[:], rq_sb[:], Act.Sin, bias=0.0, scale=W0 * TRIM)
        nc.scalar.activation(cB[:], wB[:], Act.Sin, bias=0.0, scale=W0 * TRIM)
        BlinQ = sm.tile([P, JH, Q], BF16, name="BlinQ")
        for j in range(JH):
            nc.vector.tensor_scalar(
                BlinQ[:, j, :], rq_sb[:, j, :],
                WoCK_sb[:, j, M_HARM : M_HARM + 1], None, AluOpType.mult,
            )
        gB = sm.tile([P, M_HARM, 2, JH, Q], BF16, name="gB")

        def fold(m, t, src):
            for j in range(JH):
                nc.vector.tensor_scalar(
                    gB[:, m, t, j, :], src[:, j, :],
                    WoCK_sb[:, j, m : m + 1], None, AluOpType.mult,
                )

        fold(0, 0, sB)
        fold(0, 1, cB)
        s1sB = sm.tile([P, JH, Q], BF16, name="s1sB")
        nc.vector.tensor_tensor(s1sB[:], sB[:], sB[:], AluOpType.mult)
        s2B = sm.tile([P, JH, Q], BF16, name="s2B")
        nc.vector.tensor_tensor(s2B[:], sB[:], cB[:], AluOpType.mult)
        c2B = sm.tile([P, JH, Q], BF16, name="c2B")
        nc.vector.tensor_scalar(c2B[:], s1sB[:], -2.0, 1.0, AluOpType.mult, AluOpType.add)
        fold(1, 0, s2B)
        fold(1, 1, c2B)

        # A side: wraps + the four A sins back-to-back, products chasing
        for j in range(JH):
            nc.vector.add_range_wrap(
                wA[:, j, :], rcp[:, j, :], (PI / 2) / W0, P0 / 2, P0
            )
        nc.scalar.activation(
            sA[:, 0, :], rcp[:, 0, :], Act.Sin, bias=0.0, scale=W0 * TRIM
        )
        nc.scalar.activation(
            cA[:, 0, :], wA[:, 0, :], Act.Sin, bias=0.0, scale=W0 * TRIM
        )
        nc.scalar.activation(
            sA[:, 1, :], rcp[:, 1, :], Act.Sin, bias=0.0, scale=W0 * TRIM
        )
        nc.scalar.activation(
            cA[:, 1, :], wA[:, 1, :], Act.Sin, bias=0.0, scale=W0 * TRIM
        )

        def a_products(j):
            nc.vector.tensor_tensor(
                s1sA[:, j, :], sA[:, j, :], sA[:, j, :], AluOpType.mult
            )
            nc.vector.tensor_tensor(
                s2A[:, j, :], sA[:, j, :], cA[:, j, :], AluOpType.mult
            )
            nc.vector.tensor_scalar(
                c2A[:, j, :], s1sA[:, j, :], -2.0, 1.0, AluOpType.mult, AluOpType.add
            )

        a_products(0)
        a_products(1)

        # ---- remaining logit chunks, ordered by feature readiness
        mm(onesQ[0:1, :], mrow_sb[:])               # ln(mask) rank-1
        for j in range(JH):
            mm(BlinQ[:, j, :], onesC[:])            # k*Wo.rq' broadcast over c
        for i in range(26):  # p-state bridge over the feature wait
            nc.tensor.matmul(
                tp[0:Q, 1, :], warm[:], warm[:], start=True, stop=True
            )
        mm(gB[:, 0, 1, 0, :], sA[:, 0, :])
        mm(gB[:, 0, 0, 0, :], cA[:, 0, :])
        mm(gB[:, 1, 1, 0, :], s2A[:, 0, :])
        mm(gB[:, 1, 0, 0, :], c2A[:, 0, :])
        mm(gB[:, 0, 1, 1, :], sA[:, 1, :])
        mm(gB[:, 0, 0, 1, :], cA[:, 1, :])
        mm(gB[:, 1, 1, 1, :], s2A[:, 1, :])
        mm(gB[:, 1, 0, 1, :], c2A[:, 1, :], stop=True)

        # ---- softmax tail: exp (+ masked row sums via accum_out), PE
        # transposes of the raw exp start immediately (no rowsum wait); the
        # 1/rowsum lands as a per-partition ACT scale on the final copy.
        expQ = sm.tile([Q, C], F32)
        sumQ = sm.tile([Q, 1], F32)
        nc.scalar.activation(
            expQ[:], lg[:], Act.Exp, bias=bo_sb[0:Q, 0:1], accum_out=sumQ[:]
        )
        for i in range(5):  # keep the PE p-state up through the exp wait
            nc.tensor.matmul(
                tp[0:Q, 0, :], warm[:], warm[:], start=True, stop=True
            )
        for k in range(KC):
            nc.tensor.transpose(
                tp[:, k, :], expQ[:, k * P : (k + 1) * P], ident[:]
            )
        eT_sb = sm.tile([P, KC, Q], BF16)
        nc.scalar.activation(eT_sb[:], tp[:], Act.Copy)
        # recQ chain emitted after eT so its ACT-side accumulator read does
        # not delay the eT staging on the in-order ACT queue
        recQ = sm.tile([Q, 1], F32)
        nc.vector.tensor_scalar_add(recQ[:], sumQ[:], float(EPS))
        nc.vector.reciprocal(recQ[:], recQ[:])
        w_sb = sm.tile([Q, C], BF16)
        nc.vector.tensor_scalar(
            w_sb[:], expQ[:], recQ[:, 0:1], None, AluOpType.mult
        )
        nc.sync.dma_start(wts_d.ap()[:, :], w_sb[:])
        ou = ps_ou.tile([Q, D], F32)
        for i in range(3):  # bridge the eT staging wait on a warm PE
            nc.tensor.matmul(
                tp[0:Q, 1, :], warm[:], warm[:], start=True, stop=True
            )
        for k in range(KC):
            nc.tensor.matmul(
                ou[:], eT_sb[:, k, :], ctx_sb[:, k, :],
                start=(k == 0), stop=(k == KC - 1),
            )
        out_sb = sm.tile([Q, D], BF16)
        nc.scalar.activation(out_sb[:], ou[:], Act.Copy, scale=recQ[:, 0:1])
        nc.sync.dma_start(out_d.ap()[:, :], out_sb[:])

    nc.compile()
    return nc


def _chunked(a, p=P):
    """[N*p, cols] -> [p, N*cols]: row k*p+i lands at [i, k*cols:(k+1)*cols]."""
    n = a.shape[0] // p
    return np.ascontiguousarray(
        a.reshape(n, p, a.shape[1]).transpose(1, 0, 2).reshape(p, n * a.shape[1])
    )


def make_in_maps(query, context, mask, W_c, b_c, W_q, W_o):
    import ml_dtypes
    f32 = np.float32
    bf16 = ml_dtypes.bfloat16
    WqT = _chunked(np.asarray(W_q, f32).T.astype(bf16))
    WcT = _chunked(np.asarray(W_c, f32).T.astype(bf16))
    qbulk_w = WqT  # appended after qT per batch below
    Wo2 = np.asarray(W_o, f32).reshape(JH, P).T  # (P, JH)
    # the m=2 product feature is sin2/2, so its fold carries 2x
    cols = [f32(c) for c in CS]
    cols[1] = f32(2.0) * cols[1]
    cols.append(f32(K_LIN))
    WoCK = np.stack([Wo2 * c for c in cols], axis=2)  # (P, JH, M+1)
    bc2 = np.asarray(b_c, f32).reshape(JH, P).T[:, :, None]  # (P, JH, 1)
    WoCKB = np.ascontiguousarray(
        np.concatenate([WoCK, bc2], axis=2).astype(f32)
    )  # (P, JH, M+2)
    u = f32(K_LIN) * (np.asarray(W_o, f32) @ np.asarray(W_c, f32))  # (D,)
    u2 = np.ascontiguousarray(u.reshape(KD, P).T.astype(f32))  # (P, KD)
    in_maps = []
    for b in range(B):
        mrow = np.asarray(mask[b], f32)
        mbr = np.maximum(np.log(np.maximum(mrow, 1e-300)), -50.0)
        in_maps.append(
            {
                "cbulk": np.ascontiguousarray(np.concatenate(
                    [WcT, _chunked(np.asarray(context[b], f32).T.astype(bf16))],
                    axis=1,
                )),
                "qbulk": np.ascontiguousarray(np.concatenate(
                    [_chunked(np.asarray(query[b], f32).T.astype(bf16)), qbulk_w],
                    axis=1,
                )),
                "ctx": _chunked(np.asarray(context[b], bf16)),
                "mrow": np.ascontiguousarray(mbr.reshape(1, C).astype(bf16)),
                "WoCK": WoCKB,
                "u2": u2,
            }
        )
    return in_maps


def kernel(query, context, mask, W_c, b_c, W_q, W_o, b_o):
    from concourse.bass_utils import run_bass_kernel_spmd

    nc = _build_program(float(np.asarray(b_o)))
    in_maps = make_in_maps(query, context, mask, W_c, b_c, W_q, W_o)
    res = run_bass_kernel_spmd(nc, in_maps, list(range(N_CORES))).results
    out = np.stack([np.asarray(res[b]["out"], np.float32) for b in range(B)])
    wts = np.stack([np.asarray(res[b]["wts"], np.float32) for b in range(B)])
    return out, wts


# revision 34
# speedup vs baseline: 1.0339x; 1.0082x over previous
"""Bass/Tile TRN2 kernel for nn_Attention (additive/Bahdanau-style attention).

reference math per batch b:
  res_q = query[b] @ W_q.T                      (Q, H)
  res_c = context[b] @ W_c.T + b_c              (C, H)
  logit[q,c] = sum_h W_o[h]*tanh(res_c[c,h] + res_q[q,h]) + b_o
  w = mask * exp(logit); weights = w / (sum_c w + eps)
  out = weights @ context[b]

The (Q,C,H) tanh grid is never materialized. tanh is replaced by an
M-term harmonic sine series  tanh(x) ~= k*x + sum_m c_m sin(m*w0*x), and
the angle-addition identity factorizes each term so the whole logit is
ONE PE contraction:  logit[q,c] = sum_f B_f[.,q] * A_f[.,c]   where
 - the harmonic features contract over (h, m, sin/cos) chunks of 128:
   only the fundamental sin/cos touch the ACT Sin table (cos through one
   DVE add_range_wrap with the pi/2 phase folded into the wrap shift);
   harmonic 2 comes from double-angle products, with the 2x of
   s2'=sin2/2 folded into the host-side W_o*c_m coefficients,
 - b_c folds into the rq staging copy (per-partition DVE bias), so the
   A(context) side is bias-free,
 - the linear k*Wo.rc term contracts over d against ctxT directly via
   the host-precomputed u = k*(Wo @ W_c), so res_c is never staged to
   bf16 (the sins read the f32 PSUM in place),
 - the linear k*Wo.rq' term broadcasts over c via a ones moving tile,
 - a rank-1 ln(mask) chunk folds the mask into the logit so exp's
   accum_out directly yields the masked softmax denominator.

The PE transposes of the raw exp start right after exp (no rowsum
dependency); 1/rowsum lands as a per-partition ACT scale on the final
output copy.

All matmul operands are bf16; PSUM accumulation stays f32.  Every bulk
input is pre-rearranged on the host into a [128, n*cols] partition-major
layout so each DMA moves few large contiguous lines.  Both HW-DGE
queues share one ~200GB/s DMA engine, so bulk rides the sync queue with
the CONTEXT side first: the critical chain is ctxT -> res_c -> sins ->
products -> last contraction chunks -> exp -> out, while the small
query side streams later and slots into the gaps.

Sharding: data-parallel over batch B=8 across the 8 NeuronCores.
"""

import numpy as np

B, Q, C, D, H = 8, 64, 512, 512, 256
EPS = 1e-5
P = 128
KD = D // P   # 4 chunks of the contraction dim d
KC = C // P   # 4 chunks of the context dim c
JH = H // P   # 2 chunks of the hidden dim h
N_CORES = 8

# tanh(x) ~= K_LIN*x + sum_m CS[m]*sin(m*W0*x), fitted on [-4.7, 4.7]
M_HARM = 2
W0 = 1.05
K_LIN = 0.3266410020214013
CS = [0.4380670801317152, 0.07800815282640118]
PI = float(np.pi)
TRIM = 1.0 - 5e-7    # keeps |w0*x| strictly inside the Sin table domain


def _build_program(b_o_val: float):
    import concourse.bacc as bacc
    import concourse.mybir as mybir
    import concourse.tile as tile
    from concourse.alu_op_type import AluOpType
    from concourse import masks
    from contextlib import ExitStack

    F32 = mybir.dt.float32
    BF16 = mybir.dt.bfloat16
    Act = mybir.ActivationFunctionType

    nc = bacc.Bacc("TRN2", target_bir_lowering=False, debug=False)

    # all bulk inputs arrive pre-rearranged: [P, k*cols], partition-major
    # cbulk = [WcT (KD*H) | ctxT (KD*C)], qbulk = [qT (KD*Q) | WqT (KD*H)]:
    # packing lets the whole context stream ride TWO sync triggers and the
    # query stream ONE, instead of five serialized trigger issues
    cbulk_d = nc.dram_tensor("cbulk", [P, KD * (H + C)], BF16, kind="ExternalInput")
    qbulk_d = nc.dram_tensor("qbulk", [P, KD * (Q + H)], BF16, kind="ExternalInput")
    ctx_d = nc.dram_tensor("ctx", [P, KC * D], BF16, kind="ExternalInput")
    mrow_d = nc.dram_tensor("mrow", [1, C], BF16, kind="ExternalInput")
    # cols 0..M-1: W_o * c_m folds (2x on m=2 for the halved sin2 product);
    # col M: W_o * k_lin; col M+1: b_c
    WoCK_d = nc.dram_tensor("WoCK", [P, JH, M_HARM + 2], F32, kind="ExternalInput")
    # u2[p, k] = (k_lin * W_o @ W_c)[k*128+p]: the linear context term
    # contracts over d against ctxT directly
    u2_d = nc.dram_tensor("u2", [P, KD], F32, kind="ExternalInput")
    out_d = nc.dram_tensor("out", [Q, D], BF16, kind="ExternalOutput")
    wts_d = nc.dram_tensor("wts", [Q, C], BF16, kind="ExternalOutput")

    with tile.TileContext(nc) as tc, ExitStack() as ctx:
        const = ctx.enter_context(tc.tile_pool(name="const", bufs=1))
        sm = ctx.enter_context(tc.tile_pool(name="sm", bufs=1))
        ps_rc = ctx.enter_context(tc.tile_pool(name="ps_rc", bufs=1, space="PSUM"))
        ps_rq = ctx.enter_context(tc.tile_pool(name="ps_rq", bufs=1, space="PSUM"))
        ps_lg = ctx.enter_context(tc.tile_pool(name="ps_lg", bufs=1, space="PSUM"))
        ps_tp = ctx.enter_context(tc.tile_pool(name="ps_tp", bufs=1, space="PSUM"))
        ps_ou = ctx.enter_context(tc.tile_pool(name="ps_ou", bufs=1, space="PSUM"))

        # ---- input DMAs: context side first on the sync queue
        cbulk_sb = const.tile([P, KD * (H + C)], BF16)
        qbulk_sb = const.tile([P, KD * (Q + H)], BF16)
        WoCK_sb = const.tile([P, JH, M_HARM + 2], F32)
        u2_sb = const.tile([P, KD], F32)
        mrow_sb = const.tile([1, C], BF16)
        CB0 = KD * H   # ctxT base inside cbulk
        QB0 = KD * Q   # WqT base inside qbulk

        def WcTa(k, hs):
            return cbulk_sb[:, k * H + hs.start : k * H + hs.stop]

        def ctxTa(k):
            return cbulk_sb[:, CB0 + k * C : CB0 + (k + 1) * C]

        def qTa(k):
            return qbulk_sb[:, k * Q : (k + 1) * Q]

        def WqTa(k, hs):
            return qbulk_sb[:, QB0 + k * H + hs.start : QB0 + k * H + hs.stop]

        split = CB0 + 2 * C
        nc.sync.dma_start(cbulk_sb[:, 0:split], cbulk_d.ap()[:, 0:split])
        nc.sync.dma_start(
            cbulk_sb[:, split : KD * (H + C)], cbulk_d.ap()[:, split : KD * (H + C)]
        )
        nc.sync.dma_start(qbulk_sb[:], qbulk_d.ap())
        nc.scalar.dma_start(WoCK_sb[:], WoCK_d.ap())
        nc.scalar.dma_start(u2_sb[:], u2_d.ap())
        nc.scalar.dma_start(mrow_sb[:], mrow_d.ap())
        # ctx (only needed by the final weights @ ctx matmul) is triggered
        # after the res matmuls are emitted: readers of any earlier tensor
        # conservatively wait on all previously-issued DMAs
        ctx_sb = const.tile([P, KC, D], BF16)

        onesC = const.tile([P, C], BF16)
        nc.gpsimd.memset(onesC[:], 1.0)
        onesQ = const.tile([P, Q], BF16)
        nc.gpsimd.memset(onesQ[:], 1.0)
        ident = const.tile([Q, Q], F32)
        masks.make_identity(nc, ident[:])
        bo_sb = const.tile([P, 1], F32)
        nc.vector.memset(bo_sb[:], float(b_o_val))
        # dummy first ACT op: forces the trig table (which also contains
        # Identity/Copy/Square) to be the one loaded during the DMA stream
        warm = const.tile([P, Q], BF16)
        nc.vector.memset(warm[:], 0.25)
        sinwarm = sm.tile([1, 1], BF16, name="sinwarm")
        nc.scalar.activation(sinwarm[:], warm[0:1, 0:1], Act.Sin, bias=0.0, scale=W0)
        # Bu[p, k, q] = u2[p, k] broadcast over q (stationary for the
        # linear-context chunks)
        Bu = sm.tile([P, KD, Q], BF16, name="Bu")
        for k in range(KD):
            nc.vector.tensor_scalar(
                Bu[:, k, :], onesQ[:], u2_sb[:, k : k + 1], None, AluOpType.mult
            )

        # ---- PE warmup junk while the context stream lands; then res_c
        # (k-outer, chasing the two ctxT DMA chunks) interleaved with the
        # linear-context lg chunks, then res_q as soon as its inputs land.
        rcp = ps_rc.tile([P, JH, C], F32)
        rqp = ps_rq.tile([P, JH, Q], F32)
        tp = ps_tp.tile([P, KC, Q], F32)
        lg = ps_lg.tile([Q, C], F32)
        for i in range(56):
            nc.tensor.matmul(
                tp[0:Q, 0, :], warm[:], warm[:], start=True, stop=True
            )
        first = dict(v=True)

        def mm(bt, at, stop=False):
            nc.tensor.matmul(lg[:], bt, at, start=first["v"], stop=stop)
            first["v"] = False

        for k in range(KD):
            for j in range(JH):
                hs = slice(j * P, (j + 1) * P)
                nc.tensor.matmul(
                    rcp[:, j, :], WcTa(k, hs), ctxTa(k),
                    start=(k == 0), stop=(k == KD - 1),
                )
            if k == 1:
                # res_q slots into the ctxT-k23 DMA wait: the B chain then
                # runs on ACT/DVE before the A sins are even ready
                for j in range(JH):
                    hs = slice(j * P, (j + 1) * P)
                    for kq in range(KD):
                        nc.tensor.matmul(
                            rqp[:, j, :], WqTa(kq, hs), qTa(kq),
                            start=(kq == 0), stop=(kq == KD - 1),
                        )
        # linear-context chunks AFTER res_c: they fill the PE during the
        # ACT sin phase instead of delaying res_c's last matmul
        for k in range(KD):
            mm(Bu[:, k, :], ctxTa(k))               # k_lin*Wo.rc over d
        nc.sync.dma_start(ctx_sb[:], ctx_d.ap())

        # ---- A side (context, 512 cols): the sins read the res_c PSUM
        # directly, split per h-chunk j so the PE's m0 chunks start one
        # sin earlier; products on the DVE.
        P0 = 2.0 * PI / W0
        wA = sm.tile([P, JH, C], F32, name="wA")
        sA = sm.tile([P, JH, C], BF16, name="sA")
        cA = sm.tile([P, JH, C], BF16, name="cA")
        s1sA = sm.tile([P, JH, C], BF16, name="s1sA")
        s2A = sm.tile([P, JH, C], BF16, name="s2A")
        c2A = sm.tile([P, JH, C], BF16, name="c2A")
        # B side first: it only needs res_q, which completed mid-res_c,
        # so the whole B chain clears ACT/DVE before the A sins are ready
        rq_sb = sm.tile([P, JH, Q], BF16, name="rq_sb")
        for j in range(JH):
            nc.vector.tensor_scalar(
                rq_sb[:, j, :], rqp[:, j, :],
                WoCK_sb[:, j, M_HARM + 1 : M_HARM + 2], None, AluOpType.add,
            )
        wB = sm.tile([P, JH, Q], F32, name="wB")
        nc.vector.add_range_wrap(wB[:], rq_sb[:], (PI / 2) / W0, P0 / 2, P0)
        sB = sm.tile([P, JH, Q], BF16, name="sB")
        cB = sm.tile([P, JH, Q], BF16, name="cB")
        nc.scalar.activation(sB[:], rq_sb[:], Act.Sin, bias=0.0, scale=W0 * TRIM)
        nc.scalar.activation(cB[:], wB[:], Act.Sin, bias=0.0, scale=W0 * TRIM)
        BlinQ = sm.tile([P, JH, Q], BF16, name="BlinQ")
        for j in range(JH):
            nc.vector.tensor_scalar(
                BlinQ[:, j, :], rq_sb[:, j, :],
                WoCK_sb[:, j, M_HARM : M_HARM + 1], None, AluOpType.mult,
            )
        gB = sm.tile([P, M_HARM, 2, JH, Q], BF16, name="gB")

        def fold(m, t, src):
            for j in range(JH):
                nc.vector.tensor_scalar(
                    gB[:, m, t, j, :], src[:, j, :],
                    WoCK_sb[:, j, m : m + 1], None, AluOpType.mult,
                )

        fold(0, 0, sB)
        fold(0, 1, cB)
        s1sB = sm.tile([P, JH, Q], BF16, name="s1sB")
        nc.vector.tensor_tensor(s1sB[:], sB[:], sB[:], AluOpType.mult)
        s2B = sm.tile([P, JH, Q], BF16, name="s2B")
        nc.vector.tensor_tensor(s2B[:], sB[:], cB[:], AluOpType.mult)
        c2B = sm.tile([P, JH, Q], BF16, name="c2B")
        nc.vector.tensor_scalar(c2B[:], s1sB[:], -2.0, 1.0, AluOpType.mult, AluOpType.add)
        fold(1, 0, s2B)
        fold(1, 1, c2B)

        # A side: wraps + the four A sins back-to-back, products chasing
        for j in range(JH):
            nc.vector.add_range_wrap(
                wA[:, j, :], rcp[:, j, :], (PI / 2) / W0, P0 / 2, P0
            )
        nc.scalar.activation(
            sA[:, 0, :], rcp[:, 0, :], Act.Sin, bias=0.0, scale=W0 * TRIM
        )
        nc.scalar.activation(
            cA[:, 0, :], wA[:, 0, :], Act.Sin, bias=0.0, scale=W0 * TRIM
        )
        nc.scalar.activation(
            sA[:, 1, :], rcp[:, 1, :], Act.Sin, bias=0.0, scale=W0 * TRIM
        )
        nc.scalar.activation(
            cA[:, 1, :], wA[:, 1, :], Act.Sin, bias=0.0, scale=W0 * TRIM
        )

        def a_products(j):
            nc.vector.tensor_tensor(
                s1sA[:, j, :], sA[:, j, :], sA[:, j, :], AluOpType.mult
            )
            nc.vector.tensor_tensor(
                s2A[:, j, :], sA[:, j, :], cA[:, j, :], AluOpType.mult
            )
            nc.vector.tensor_scalar(
                c2A[:, j, :], s1sA[:, j, :], -2.0, 1.0, AluOpType.mult, AluOpType.add
            )

        a_products(0)
        a_products(1)

        # ---- remaining logit chunks, ordered by feature readiness
        mm(onesQ[0:1, :], mrow_sb[:])               # ln(mask) rank-1
        for j in range(JH):
            mm(BlinQ[:, j, :], onesC[:])            # k*Wo.rq' broadcast over c
        for i in range(26):  # p-state bridge over the feature wait
            nc.tensor.matmul(
                tp[0:Q, 1, :], warm[:], warm[:], start=True, stop=True
            )
        mm(gB[:, 0, 1, 0, :], sA[:, 0, :])
        mm(gB[:, 0, 0, 0, :], cA[:, 0, :])
        mm(gB[:, 1, 1, 0, :], s2A[:, 0, :])
        mm(gB[:, 1, 0, 0, :], c2A[:, 0, :])
        mm(gB[:, 0, 1, 1, :], sA[:, 1, :])
        mm(gB[:, 0, 0, 1, :], cA[:, 1, :])
        mm(gB[:, 1, 1, 1, :], s2A[:, 1, :])
        mm(gB[:, 1, 0, 1, :], c2A[:, 1, :], stop=True)

        # ---- softmax tail: exp (+ masked row sums via accum_out), PE
        # transposes of the raw exp start immediately (no rowsum wait); the
        # 1/rowsum lands as a per-partition ACT scale on the final copy.
        expQ = sm.tile([Q, C], F32)
        sumQ = sm.tile([Q, 1], F32)
        nc.scalar.activation(
            expQ[:], lg[:], Act.Exp, bias=bo_sb[0:Q, 0:1], accum_out=sumQ[:]
        )
        for i in range(18):  # keep the PE p-state up through the exp wait
            nc.tensor.matmul(
                tp[0:Q, 0, :], warm[:], warm[:], start=True, stop=True
            )
        for k in range(KC):
            nc.tensor.transpose(
                tp[:, k, :], expQ[:, k * P : (k + 1) * P], ident[:]
            )
        eT_sb = sm.tile([P, KC, Q], BF16)
        nc.scalar.activation(eT_sb[:], tp[:], Act.Copy)
        # recQ chain emitted after eT so its ACT-side accumulator read does
        # not delay the eT staging on the in-order ACT queue
        recQ = sm.tile([Q, 1], F32)
        nc.vector.tensor_scalar_add(recQ[:], sumQ[:], float(EPS))
        nc.vector.reciprocal(recQ[:], recQ[:])
        w_sb = sm.tile([Q, C], BF16)
        nc.vector.tensor_scalar(
            w_sb[:], expQ[:], recQ[:, 0:1], None, AluOpType.mult
        )
        nc.sync.dma_start(wts_d.ap()[:, :], w_sb[:])
        ou = ps_ou.tile([Q, D], F32)
        for i in range(8):  # bridge the eT staging wait on a warm PE
            nc.tensor.matmul(
                tp[0:Q, 1, :], warm[:], warm[:], start=True, stop=True
            )
        for k in range(KC):
            nc.tensor.matmul(
                ou[:], eT_sb[:, k, :], ctx_sb[:, k, :],
                start=(k == 0), stop=(k == KC - 1),
            )
        out_sb = sm.tile([Q, D], BF16)
        nc.scalar.activation(out_sb[:], ou[:], Act.Copy, scale=recQ[:, 0:1])
        nc.sync.dma_start(out_d.ap()[:, :], out_sb[:])

    nc.compile()
    return nc


def _chunked(a, p=P):
    """[N*p, cols] -> [p, N*cols]: row k*p+i lands at [i, k*cols:(k+1)*cols]."""
    n = a.shape[0] // p
    return np.ascontiguousarray(
        a.reshape(n, p, a.shape[1]).transpose(1, 0, 2).reshape(p, n * a.shape[1])
    )


def make_in_maps(query, context, mask, W_c, b_c, W_q, W_o):
    import ml_dtypes
    f32 = np.float32
    bf16 = ml_dtypes.bfloat16
    WqT = _chunked(np.asarray(W_q, f32).T.astype(bf16))
    WcT = _chunked(np.asarray(W_c, f32).T.astype(bf16))
    qbulk_w = WqT  # appended after qT per batch below
    Wo2 = np.asarray(W_o, f32).reshape(JH, P).T  # (P, JH)
    # the m=2 product feature is sin2/2, so its fold carries 2x
    cols = [f32(c) for c in CS]
    cols[1] = f32(2.0) * cols[1]
    cols.append(f32(K_LIN))
    WoCK = np.stack([Wo2 * c for c in cols], axis=2)  # (P, JH, M+1)
    bc2 = np.asarray(b_c, f32).reshape(JH, P).T[:, :, None]  # (P, JH, 1)
    WoCKB = np.ascontiguousarray(
        np.concatenate([WoCK, bc2], axis=2).astype(f32)
    )  # (P, JH, M+2)
    u = f32(K_LIN) * (np.asarray(W_o, f32) @ np.asarray(W_c, f32))  # (D,)
    u2 = np.ascontiguousarray(u.reshape(KD, P).T.astype(f32))  # (P, KD)
    in_maps = []
    for b in range(B):
        mrow = np.asarray(mask[b], f32)
        mbr = np.maximum(np.log(np.maximum(mrow, 1e-300)), -50.0)
        in_maps.append(
            {
                "cbulk": np.ascontiguousarray(np.concatenate(
                    [WcT, _chunked(np.asarray(context[b], f32).T.astype(bf16))],
                    axis=1,
                )),
                "qbulk": np.ascontiguousarray(np.concatenate(
                    [_chunked(np.asarray(query[b], f32).T.astype(bf16)), qbulk_w],
                    axis=1,
                )),
                "ctx": _chunked(np.asarray(context[b], bf16)),
                "mrow": np.ascontiguousarray(mbr.reshape(1, C).astype(bf16)),
                "WoCK": WoCKB,
                "u2": u2,
            }
        )
    return in_maps


def kernel(query, context, mask, W_c, b_c, W_q, W_o, b_o):
    from concourse.bass_utils import run_bass_kernel_spmd

    nc = _build_program(float(np.asarray(b_o)))
    in_maps = make_in_maps(query, context, mask, W_c, b_c, W_q, W_o)
    res = run_bass_kernel_spmd(nc, in_maps, list(range(N_CORES))).results
    out = np.stack([np.asarray(res[b]["out"], np.float32) for b in range(B)])
    wts = np.stack([np.asarray(res[b]["wts"], np.float32) for b in range(B)])
    return out, wts
